# revision 1
# baseline (speedup 1.0000x reference)
"""Trainium2 kernel for nn_DenseGeneral fp8-qdq forward.

Reference computes: out = qdq_e4m3(inputs) @ qdq_e4m3(kernel) + bf16_round(bias)
(forward pass of fp8-aware DenseGeneral; scale/amax updates only live in the
custom_vjp residuals and do not affect the forward output).

Strategy:
- Host: quantize inputs/kernel to e4m3 exactly as the reference does (scales are
  ones in this problem, but general scales are folded back into the output).
  OCP e4m3fn bit patterns == TRN FP8_EXP4 for |v| <= 240, which holds here.
- Shard 4-way over rows of inp_mat (data parallel) x 2-way over kernel columns
  (tensor parallel) = 8 NeuronCores.
- Device: pure fp8 matmul (DoubleRow perf mode) with fp32 PSUM accumulation via
  the production matmul_tile_kernel; f32 output.
- Host: gather, apply scale product + bias, reshape.
"""

import numpy as np
import ml_dtypes

P = 128
B, S, D, F = 4, 2048, 2048, 8192
M = B * S  # 8192 rows of inp_mat
GRID_M, GRID_N = 4, 2
M_LOC = M // GRID_M  # 2048
N_LOC = F // GRID_N  # 4096
N_CORES = 8

_PROGRAM = None  # (nc, kxm_name, kxn_name, mxn_name)


def _build_program():
    global _PROGRAM
    if _PROGRAM is not None:
        return _PROGRAM
    import concourse.bacc as bacc
    import concourse.mybir as mybir
    import concourse.tile as tile
    from concourse.kernels.tile_matmul import matmul_tile_kernel

    nc = bacc.Bacc("TRN2", target_bir_lowering=False, debug=False)
    with tile.TileContext(nc) as tc:
        with tc.tile_pool(name="dram", bufs=1, space="DRAM") as dram:
            kxm = dram.tile((P, D // P, M_LOC), mybir.dt.float8e4, kind="ExternalInput")
            kxn = dram.tile((P, D // P, N_LOC), mybir.dt.float8e4, kind="ExternalInput")
            mxn = dram.tile(
                (P, M_LOC // P, N_LOC), mybir.dt.float32, kind="ExternalOutput"
            )
            matmul_tile_kernel(tc, kxm[:], kxn[:], mxn[:])
    nc.compile()
    _PROGRAM = (nc, kxm.name, kxn.name, mxn.name)
    return _PROGRAM


def _qdq_e4m3(x, scale):
    """fp32 -> e4m3 with the reference's scale/clip semantics; TRN-fp8 view."""
    if scale != 1.0:
        x = x / np.float32(scale)
    q = np.clip(x, -448.0, 448.0).astype(ml_dtypes.float8_e4m3fn)
    return q.view(ml_dtypes.float8_e4m3)


def _to_partition_major(a):
    """[R, C] -> [P, R//P, C] with element [p, r, c] = a[r*P + p, c]."""
    R, C = a.shape
    return np.ascontiguousarray(a.reshape(R // P, P, C).transpose(1, 0, 2))


def kernel(
    inputs,
    kernel,
    bias,
    input_scale,
    kernel_scale,
    output_grad_scale,
    input_amax_history,
    kernel_amax_history,
    output_grad_amax_history,
):
    from concourse.bass_utils import run_bass_kernel_spmd

    nc, kxm_name, kxn_name, mxn_name = _build_program()

    x = np.asarray(inputs, dtype=np.float32).reshape(M, D)
    w = np.asarray(kernel, dtype=np.float32)
    s_in = float(np.asarray(input_scale).reshape(-1)[0])
    s_k = float(np.asarray(kernel_scale).reshape(-1)[0])

    xq = _qdq_e4m3(x, s_in)  # [M, D] fp8
    wq = _qdq_e4m3(w, s_k)  # [D, F] fp8

    # Per-shard DRAM images (shared across cores where slices coincide)
    kxm_shards = []
    for mi in range(GRID_M):
        xT = np.ascontiguousarray(xq[mi * M_LOC : (mi + 1) * M_LOC, :].T)  # [D, M_LOC]
        kxm_shards.append(_to_partition_major(xT))
    kxn_shards = []
    for ni in range(GRID_N):
        ws = wq[:, ni * N_LOC : (ni + 1) * N_LOC]  # [D, N_LOC]
        kxn_shards.append(_to_partition_major(np.ascontiguousarray(ws)))

    in_maps = []
    for c in range(N_CORES):
        mi, ni = divmod(c, GRID_N)
        in_maps.append({kxm_name: kxm_shards[mi], kxn_name: kxn_shards[ni]})

    res = run_bass_kernel_spmd(nc, in_maps, core_ids=list(range(N_CORES)))

    out = np.empty((M, F), dtype=np.float32)
    for c in range(N_CORES):
        mi, ni = divmod(c, GRID_N)
        block = res.results[c][mxn_name]  # [P, M_LOC//P, N_LOC]
        out[mi * M_LOC : (mi + 1) * M_LOC, ni * N_LOC : (ni + 1) * N_LOC] = (
            block.transpose(1, 0, 2).reshape(M_LOC, N_LOC)
        )

    sprod = s_in * s_k
    if sprod != 1.0:
        out *= np.float32(sprod)

    b = np.asarray(bias, dtype=np.float32)
    b = b.astype(ml_dtypes.bfloat16).astype(np.float32)
    if np.any(b):
        out += b[None, :]

    return out.reshape(B, S, F)
